# revision 28
# baseline (speedup 1.0000x reference)
"""CapsNet dynamic-routing kernel for TRN2, 8 NeuronCores, data-parallel over batch.

Reference computes u_hat = u_vecs @ W ([64,1024,2048], 137 GFLOP) then 3 routing
iterations over it. This kernel never materializes u_hat: every routing
contraction is re-associated through u_vecs / W directly:

  v[b,n,:]   = sum_i c[b,n,i] u_vecs[b,i,:]          (contract i, 1024)
  pre[b,n,:] = v[b,n,:] @ W_n                         (contract k, per capsule)
  outputs    = squash(pre)
  w2[b,n,:]  = outputs[b,n,:] @ W_n^T                 (contract d, per capsule)
  b[b,:,i]   = w2[b,:,:] @ u_vecs[b,i,:]^T            (contract k, 512)

~15x fewer FLOPs than materializing u_hat. fp16 operands / fp32 accumulation.

Per core: 8 batches. Host ships fp16 casts, a transposed copy of u_vecs, and
the iter-0 column sums (softmax(0) is uniform), so no on-chip u_vecs
transposes are needed.
"""

import numpy as np

ROUTINGS = 3
NC_CAP = 32
DC = 64
EPS = 1e-7
N_CORES = 8
B, N_IN, D_IN = 64, 1024, 512
B_LOC = B // N_CORES  # 8

_cached = {}


def _build_program():
    import concourse.bass as bass
    import concourse.tile as tile
    from concourse import bacc, mybir

    f16 = mybir.dt.float16
    f32 = mybir.dt.float32
    ADD = mybir.AluOpType.add
    AX = mybir.AxisListType.X
    AF = mybir.ActivationFunctionType

    nc = bacc.Bacc("TRN2", target_bir_lowering=False, debug=False,
                   num_devices=N_CORES)

    u16_d = nc.dram_tensor("u16", [B_LOC, N_IN, D_IN], f16, kind="ExternalInput").ap()
    ut16_d = nc.dram_tensor("ut16", [B_LOC, D_IN, N_IN], f16, kind="ExternalInput").ap()
    w16_d = nc.dram_tensor("w16", [D_IN, NC_CAP * DC], f16, kind="ExternalInput").ap()
    # WT packed: [128=(tau,d), 16=(m,g), 512] ; capsule n = 8m + 4tau + g
    wt16_d = nc.dram_tensor("wt16", [128, 16, D_IN], f16, kind="ExternalInput").ap()
    # s32T: column sums of u_vecs / 32, transposed: [128=(k%128), 4=(k//128), 8=b']
    s32t_d = nc.dram_tensor("s32t", [128, 4, B_LOC], f16, kind="ExternalInput").ap()
    ident_d = nc.dram_tensor("ident", [128, 128], f16, kind="ExternalInput").ap()
    out_d = nc.dram_tensor("out", [B_LOC, NC_CAP, DC], f32, kind="ExternalOutput").ap()

    with tile.TileContext(nc) as tc:
        with (
            tc.tile_pool(name="big", bufs=1) as big,
            tc.tile_pool(name="work", bufs=1) as work,
            tc.tile_pool(name="ps", bufs=2, space="PSUM") as psp,
            tc.tile_pool(name="ps1", bufs=1, space="PSUM") as psp1,
            tc.tile_pool(name="ps3", bufs=3, space="PSUM") as psp3,
        ):
            U = big.tile([128, B_LOC, 8, D_IN], f16, tag="U")        # (i%128),(b),(i//128),(k)
            UT = big.tile([128, B_LOC, 4, N_IN], f16, tag="UT")      # (k%128),(b),(k//128),(i)
            W16 = big.tile([128, 4, NC_CAP * DC], f16, tag="W16")    # (k%128),(k//128),(n d)
            WT16 = big.tile([128, 16, D_IN], f16, tag="WT16")
            S32T = work.tile([128, 4, B_LOC], f16, tag="S32T")
            IDENT = work.tile([128, 128], f16, tag="IDENT")

            vT_all = work.tile([128, 4, B_LOC, NC_CAP], f16, tag="vT")    # (k%128),(j),(b),(n)
            w2T_all = work.tile([128, 4, B_LOC, NC_CAP], f16, tag="w2T")  # (k%128),(j),(b),(n)
            c_sb = work.tile([128, B_LOC, 8, NC_CAP], f16, tag="c")       # (i%128),(b),(t),(n)
            e_sb = work.tile([128, B_LOC, 8, NC_CAP], f16, tag="e")
            # masked outputs^T for paired w2 matmuls: [(tau d), 16P=(m g), (tau', b)]
            L_sb = work.tile([128, 16, 2, B_LOC], f16, tag="L")
            z_sb = work.tile([128, B_LOC, 8], f32, tag="z")
            r_sb = work.tile([128, B_LOC, 8], f32, tag="r")
            outp16 = work.tile([128, 8, DC], f16, tag="outp16")           # (32g+b),(T),(d)
            outT = work.tile([128, 4, 128], f16, tag="outT")              # (tau d),(m),(32g+b)
            nrm = work.tile([128, 8], f32, tag="nrm")
            sq2 = work.tile([128, 8, DC], f32, tag="sq2")
            sq = work.tile([128, 8], f32, tag="sq")
            scl = work.tile([128, 8], f32, tag="scl")
            outp32 = work.tile([128, 8, DC], f32, tag="outp32")
            eps_t = work.tile([128, 1], f32, tag="eps")
            nc.gpsimd.memset(eps_t[:], EPS)

            # ---- loads ----
            # Single sync ring (multi-ring spreads measured slower), but UT/U
            # interleaved per batch: UT[b] feeds the iter-0 b-update, U[b] the
            # iter-1 v matmuls, so arrival order matches consumption order.
            nc.sync.dma_start(S32T[:], s32t_d[:])
            nc.sync.dma_start(W16[:], w16_d.rearrange("(j p) z -> p j z", p=128))
            nc.sync.dma_start(WT16[:], wt16_d[:])
            nc.sync.dma_start(IDENT[:], ident_d[:])
            for b in range(B_LOC):
                nc.sync.dma_start(UT[:, b], ut16_d[b].rearrange("(j p) i -> p j i", p=128))
                nc.sync.dma_start(U[:, b], u16_d[b].rearrange("(t p) k -> p t k", p=128))

            def caps_mm_pre(pre_ps, lhsT_of):
                # pre[b', n, :]: out rows (g,b') at base 32g, cols (T,d).
                for T in range(8):
                    for g in range(4):
                        for j in range(4):
                            n = 4 * T + g
                            nc.tensor.matmul(
                                pre_ps[32 * g:32 * g + B_LOC, T],
                                lhsT_of(j, n),
                                W16[:, j, n * DC:(n + 1) * DC],
                                start=(j == 0), stop=(j == 3),
                                tile_position=(0, 32 * g),
                            )

            def squash(pre_ps, it):
                nc.scalar.activation(sq2[:], pre_ps[:], AF.Square)
                nc.vector.tensor_reduce(nrm[:], sq2[:], AX, ADD)
                nc.scalar.activation(sq[:], nrm[:], AF.Sqrt, bias=eps_t[:])
                nc.vector.reciprocal(scl[:], sq[:])
                dst = outp16 if it < ROUTINGS - 1 else outp32
                nc.vector.tensor_mul(dst[:], pre_ps[:],
                                     scl[:].broadcast_to((128, 8, DC)))
                if it == ROUTINGS - 1:
                    dr = out_d.rearrange("b (T g) d -> g b T d", g=4)
                    for g in range(4):
                        nc.sync.dma_start(dr[g], outp32[32 * g:32 * g + B_LOC])

            def transpose_and_w2():
                # shares the "pre" slot: tp is only live after pre's readers finish
                tp_ps = psp1.tile([128, 4, 128], f16, tag="pre")
                for m in range(4):
                    nc.tensor.transpose(
                        tp_ps[:, m],
                        outp16[:, 2 * m:2 * m + 2, :].rearrange("p a b -> p (a b)"),
                        IDENT[:])
                nc.vector.tensor_copy(outT[:], tp_ps[:])
                # Build the block-diagonal mask L so one matmul covers a
                # capsule pair: L[(tau,d), P, (tau',b)] = outT iff tau==tau'.
                nc.vector.memset(L_sb[:], 0.0)
                for tau in range(2):
                    nc.vector.tensor_copy(
                        L_sb[64 * tau:64 * tau + 64, :, tau, :],
                        outT[64 * tau:64 * tau + 64, :, :]
                        .rearrange("p m (g c) -> p (m g) c", g=4)[:, :, 0:B_LOC])
                # One matmul per (capsule pair P, k' chunk j): K spans both
                # capsules' d, the zero blocks in L kill cross terms.
                w2pn = psp1.tile([128, 4, 16, 2, B_LOC], f32, tag="w2pn")
                for p in range(16):
                    for j in range(4):
                        nc.tensor.matmul(
                            w2pn[:, j, p],
                            WT16[:, p, 128 * j:128 * j + 128],
                            L_sb[:, p],
                            start=True, stop=True,
                        )
                # w2T_all[:, j, b, n] with n = 8m + 4tau + g
                w2v = w2T_all[:].rearrange("p j b (m x g) -> p x j m g b", m=4, x=2, g=4)
                for tau in range(2):
                    for j in range(4):
                        nc.vector.tensor_copy(
                            w2v[:, tau, j],
                            w2pn[:, j, :, tau].rearrange("p (m g) b -> p m g b", g=4))

            def bupdate_softmax(b):
                b_ps = psp.tile([128, 8, NC_CAP], f32, tag="b_ps")
                for t in range(8):
                    for j in range(4):
                        nc.tensor.matmul(
                            b_ps[:, t], UT[:, b, j, 128 * t:128 * t + 128],
                            w2T_all[:, j, b, :], start=(j == 0), stop=(j == 3))
                nc.scalar.activation(e_sb[:, b], b_ps[:], AF.Exp)
                nc.vector.tensor_reduce(z_sb[:, b], e_sb[:, b], AX, ADD)
                nc.vector.reciprocal(r_sb[:, b], z_sb[:, b])
                nc.vector.tensor_mul(
                    c_sb[:, b], e_sb[:, b],
                    r_sb[:, b].broadcast_to((128, 8, NC_CAP)))

            def v_matmul(b):
                vT_ps = psp3.tile([128, 4, NC_CAP], f32, tag="vT_ps")
                for j in range(4):
                    for t in range(8):
                        nc.tensor.matmul(
                            vT_ps[:, j], U[:, b, t, 128 * j:128 * j + 128],
                            c_sb[:, b, t, :], start=(t == 0), stop=(t == 7))
                nc.scalar.copy(vT_all[:, :, b, :], vT_ps[:])

            # ================= schedule =================
            for it in range(ROUTINGS):
                pre_ps = psp1.tile([128, 8, DC], f32, tag="pre")
                nc.vector.memset(pre_ps[:], 0.0)
                if it == 0:
                    with nc.named_scope(f"i{it}_pre"):
                        caps_mm_pre(pre_ps, lambda j, n: S32T[:, j, :])
                else:
                    with nc.named_scope(f"i{it}_v"):
                        for b in range(B_LOC):
                            v_matmul(b)
                    with nc.named_scope(f"i{it}_pre"):
                        caps_mm_pre(pre_ps, lambda j, n: vT_all[:, j, :, n])
                with nc.named_scope(f"i{it}_squash"):
                    squash(pre_ps, it)
                if it < ROUTINGS - 1:
                    with nc.named_scope(f"i{it}_w2"):
                        transpose_and_w2()
                    with nc.named_scope(f"i{it}_bup"):
                        for b in range(B_LOC):
                            bupdate_softmax(b)

    nc.compile()
    return nc


def _host_prep(u_vecs, W):
    u_vecs = np.asarray(u_vecs, dtype=np.float32)
    W = np.asarray(W, dtype=np.float32).reshape(D_IN, NC_CAP * DC)

    w16 = W.astype(np.float16)
    Wr = W.reshape(D_IN, NC_CAP, DC)  # [k', n, d]
    wt = np.zeros((128, 16, D_IN), dtype=np.float16)
    for m in range(4):
        for g in range(4):
            for tau in range(2):
                n = 8 * m + 4 * tau + g
                wt[64 * tau:64 * tau + 64, 4 * m + g, :] = Wr[:, n, :].T.astype(np.float16)

    ident = np.eye(128, dtype=np.float16)

    in_maps = []
    for c in range(N_CORES):
        ub = u_vecs[c * B_LOC:(c + 1) * B_LOC]  # [8, 1024, 512] fp32
        u16 = ub.astype(np.float16)
        ut16 = np.ascontiguousarray(u16.transpose(0, 2, 1))  # [8, 512, 1024]
        s = ub.sum(axis=1) / NC_CAP                           # [8, 512] fp32
        s32t = np.ascontiguousarray(
            s.T.reshape(4, 128, B_LOC).transpose(1, 0, 2)).astype(np.float16)
        in_maps.append({
            "u16": u16, "ut16": ut16, "w16": w16, "wt16": wt,
            "s32t": s32t, "ident": ident,
        })
    return in_maps


def kernel(u_vecs, W):
    from concourse.bass_utils import run_bass_kernel_spmd

    if "nc" not in _cached:
        _cached["nc"] = _build_program()
    nc = _cached["nc"]

    in_maps = _host_prep(u_vecs, W)
    res = run_bass_kernel_spmd(nc, in_maps, list(range(N_CORES)))
    out = np.concatenate([res.results[c]["out"] for c in range(N_CORES)], axis=0)
    return out.astype(np.float32)
